# revision 28
# baseline (speedup 1.0000x reference)
"""MoD router kernel for Trainium2 (8 NeuronCores, SPMD).

Problem: hidden [4, 8192, 2048] f32, router_weight [2048], router_bias [].
  logits = hidden @ w + b ; probs = sigmoid(logits)
  mask   = top-k per sequence (k = 4096 of 8192), 1.0/0.0
  aux    = 0.01 * (mean_prob_per_seq - 0.5)^2

Sharding: 8 cores = 4 sequences x 2 halves. Core c handles sequence c//2,
tokens [ (c%2)*4096, (c%2+1)*4096 ). The matvec is DMA-bound (32 MiB/core);
it runs as a fused multiply+reduce on VectorE (tensor_tensor_reduce) against
a host-replicated weight tile, with sigmoid on ScalarE and a 256-edge
count(>=edge) histogram on fixed logit-space edges accumulated per tile
(scalar_tensor_tensor) under the DMA shadow.

Top-k threshold: per-pair AllGather exchanges logits + histogram + row-sum.
The histogram gives an exact bracket [lo, lo+2^-6) containing the k-th
largest logit; in-bracket candidates are extracted with max8 per partition,
and 7 radix rounds of 64-ary counting on monotone int32 keys pin the exact
k-th largest value t*. mask = (key >= t*).
"""

import sys

sys.path.insert(0, "/opt/trn_rl_repo")

import numpy as np

import concourse.bacc as bacc
import concourse.bass as bass
import concourse.mybir as mybir
from concourse import tile
from concourse.bass_utils import run_bass_kernel_spmd

F32 = mybir.dt.float32
I32 = mybir.dt.int32
OP = mybir.AluOpType
ACT = mybir.ActivationFunctionType

B, S, D = 4, 8192, 2048
N_CORES = 8
K_FULL = S // 2          # 4096
S_LOC = S // 2           # tokens per core
N_HIST = 256             # histogram edges
EDGE_LO = -2.0           # first edge (exact dyadic)
EDGE_STEP = 2.0 ** -6    # edge spacing  (exact dyadic)
N_ROUNDS = 7             # 64-ary int radix rounds (64**7 >> 2**31)


def build_kernel(s_loc=S_LOC, d=D, k=K_FULL, chunk_tiles=4, groups=None,
                 num_devices=N_CORES, debug_dump=False):
    """Build the SPMD Bass program (identical on every core)."""
    if groups is None:
        groups = [[2 * i, 2 * i + 1] for i in range(num_devices // 2)]
    n_tiles = s_loc // 128
    n_chunks = n_tiles // chunk_tiles
    assert n_tiles % chunk_tiles == 0
    pk = 32 + 1 + 1 + N_HIST   # packed cols: logits | rowsum | pad | hist
    assert n_tiles <= 32

    nc = bacc.Bacc("TRN2", target_bir_lowering=False, debug=False,
                   num_devices=num_devices)

    hid = nc.dram_tensor("hid", [s_loc, d], F32, kind="ExternalInput")
    w_rep = nc.dram_tensor("w_rep", [128, d], F32, kind="ExternalInput")
    bias_rep = nc.dram_tensor("bias_rep", [128, 1], F32, kind="ExternalInput")
    probs_out = nc.dram_tensor("probs", [s_loc], F32, kind="ExternalOutput")
    mask_out = nc.dram_tensor("mask", [s_loc], F32, kind="ExternalOutput")
    aux_out = nc.dram_tensor("aux", [1, 1], F32, kind="ExternalOutput")

    # compile-time constants
    edges_np = (EDGE_LO + np.arange(N_HIST) * EDGE_STEP).astype(np.float32)
    edges_c = nc.inline_tensor(np.tile(edges_np, (128, 1)), "edges_c")
    iota_hist_c = nc.inline_tensor(
        np.tile(np.arange(N_HIST, dtype=np.float32), (128, 1)), "iota_hist_c")
    iota64f_c = nc.inline_tensor(
        np.tile(np.arange(64, dtype=np.float32), (128, 1)), "iota64f_c")
    iota64i_c = nc.inline_tensor(
        np.tile(np.arange(64, dtype=np.int32), (128, 1)), "iota64i_c")
    ones_c = nc.inline_tensor(np.ones((128, 128), np.float32), "ones_c")
    intc_c = nc.inline_tensor(
        np.tile(np.array([31, 0x7FFFFFFF, -0x80000000, 16, 0xFFFF],
                         dtype=np.int32), (128, 1)), "intc_c")

    with tile.TileContext(nc) as tc:
        with (
            tc.tile_pool(name="big", bufs=3) as big,
            tc.tile_pool(name="prod", bufs=2) as prodp,
            tc.tile_pool(name="persist", bufs=1) as pp,
            tc.tile_pool(name="small", bufs=2) as sp,
            tc.tile_pool(name="psum", bufs=1, space="PSUM") as psp,
            tc.tile_pool(name="dram", bufs=1, space="DRAM") as dp,
        ):
            # --- persistent SBUF state -------------------------------------
            w_sb = pp.tile([128, d], F32, tag="w_sb")
            bias_sb = pp.tile([128, 1], F32, tag="bias_sb")
            edges_sb = pp.tile([128, N_HIST], F32, tag="edges_sb")
            iota_hist = pp.tile([128, N_HIST], F32, tag="iota_hist")
            iota64f = pp.tile([128, 64], F32, tag="iota64f")
            iota64i = pp.tile([128, 64], I32, tag="iota64i")
            ones_sb = pp.tile([128, 128], F32, tag="ones_sb")
            intc = pp.tile([128, 5], I32, tag="intc")
            logits_sb = pp.tile([128, n_tiles], F32, tag="logits_sb")
            probs_sb = pp.tile([128, n_tiles], F32, tag="probs_sb")
            hist = pp.tile([128, N_HIST], F32, tag="hist")
            packed = pp.tile([128, pk], F32, tag="packed")

            nc.sync.dma_start(out=w_sb[:], in_=w_rep[:])
            nc.sync.dma_start(out=bias_sb[:], in_=bias_rep[:])
            nc.sync.dma_start(out=edges_sb[:], in_=edges_c[:])
            nc.sync.dma_start(out=iota_hist[:], in_=iota_hist_c[:])
            nc.sync.dma_start(out=iota64f[:], in_=iota64f_c[:])
            nc.sync.dma_start(out=iota64i[:], in_=iota64i_c[:])
            nc.sync.dma_start(out=ones_sb[:], in_=ones_c[:])
            nc.sync.dma_start(out=intc[:], in_=intc_c[:])
            nc.vector.memset(hist[:], 0.0)

            # --- main pass: matvec + sigmoid + histogram -------------------
            # token (chunk c, slot i, partition p) = (c*CT + i)*128 + p
            hid_r = hid.ap().rearrange("(c i p) e -> c p i e",
                                       i=chunk_tiles, p=128)
            for ci in range(n_chunks):
                hid_t = big.tile([128, chunk_tiles * d], F32, tag="hid_t")
                nc.sync.dma_start(
                    out=hid_t[:].rearrange("p (i e) -> p i e", i=chunk_tiles),
                    in_=hid_r[ci])
                for i in range(chunk_tiles):
                    t = ci * chunk_tiles + i
                    prod = prodp.tile([128, d], F32, tag="prod")
                    # fused multiply + free-dim sum: logits[:, t] = sum(h * w)
                    # (tensor_tensor_reduce hard-faults the exec unit on this
                    # stack; scalar_tensor_tensor with accum_out is the
                    # HW-verified equivalent)
                    nc.vector.scalar_tensor_tensor(
                        out=prod[:],
                        in0=hid_t[:, bass.ts(i, d)],
                        scalar=0.0,
                        in1=w_sb[:],
                        op0=OP.bypass,
                        op1=OP.mult,
                        accum_out=logits_sb[:, t:t + 1],
                    )
                    nc.scalar.activation(
                        probs_sb[:, t:t + 1], logits_sb[:, t:t + 1],
                        ACT.Sigmoid, bias=bias_sb[:, 0:1], scale=1.0)
                    # hist[p,j] += (edge_j <= logit[p])
                    nc.vector.scalar_tensor_tensor(
                        out=hist[:], in0=edges_sb[:],
                        scalar=logits_sb[:, t:t + 1], in1=hist[:],
                        op0=OP.is_le, op1=OP.add)

            # row-sum of probs -> partition-reduced, replicated
            pcol = sp.tile([128, 1], F32, tag="pcol")
            nc.vector.tensor_reduce(out=pcol[:], in_=probs_sb[:],
                                    axis=mybir.AxisListType.X, op=OP.add)
            ps_rs = psp.tile([128, 1], F32, tag="ps_rs")
            nc.tensor.matmul(ps_rs[:], ones_sb[:], pcol[:], start=True,
                             stop=True)

            # --- pack + pairwise AllGather ---------------------------------
            nc.vector.memset(packed[:], 0.0)
            nc.vector.tensor_copy(packed[:, 0:n_tiles], logits_sb[:])
            nc.scalar.copy(packed[:, 32:33], ps_rs[:])
            nc.vector.tensor_copy(packed[:, 34:34 + N_HIST], hist[:])

            cc_in = dp.tile([128, pk], F32, tag="cc_in")
            cc_out = dp.tile([2, 128, pk], F32, tag="cc_out")
            nc.sync.dma_start(out=cc_in[:], in_=packed[:])
            nc.gpsimd.collective_compute(
                "AllGather", OP.bypass, replica_groups=groups,
                ins=[cc_in[:].opt()], outs=[cc_out[:].opt()])
            gath = pp.tile([128, 2 * pk], F32, tag="gath")
            nc.sync.dma_start(
                out=gath[:].rearrange("p (b c) -> p b c", b=2),
                in_=cc_out[:].rearrange("b p c -> p b c"))
            g3 = gath[:].rearrange("p (b c) -> p b c", b=2)

            # --- merge histograms, pick bracket ----------------------------
            hist_tot = sp.tile([128, N_HIST], F32, tag="hist_tot")
            nc.vector.tensor_tensor(out=hist_tot[:], in0=g3[:, 0, 34:34 + N_HIST],
                                    in1=g3[:, 1, 34:34 + N_HIST], op=OP.add)
            ps_cnt = psp.tile([128, N_HIST], F32, tag="ps_cnt")
            nc.tensor.matmul(ps_cnt[:], ones_sb[:], hist_tot[:], start=True,
                             stop=True)
            ind = sp.tile([128, N_HIST], F32, tag="ind")
            nc.vector.tensor_scalar(out=ind[:], in0=ps_cnt[:],
                                    scalar1=float(k), scalar2=None,
                                    op0=OP.is_ge)
            jsel = sp.tile([128, N_HIST], F32, tag="jsel")
            nc.vector.tensor_tensor(out=jsel[:], in0=ind[:], in1=iota_hist[:],
                                    op=OP.mult)
            jcol = sp.tile([128, 1], F32, tag="jcol")
            nc.vector.tensor_reduce(out=jcol[:], in_=jsel[:],
                                    axis=mybir.AxisListType.X, op=OP.max)
            lo_f = sp.tile([128, 1], F32, tag="lo_f")
            nc.vector.tensor_scalar(out=lo_f[:], in0=jcol[:],
                                    scalar1=EDGE_STEP, scalar2=EDGE_LO,
                                    op0=OP.mult, op1=OP.add)
            hi_f = sp.tile([128, 1], F32, tag="hi_f")
            nc.vector.tensor_scalar_add(out=hi_f[:], in0=lo_f[:],
                                        scalar1=EDGE_STEP)

            # --- in-bracket candidates via max8 ----------------------------
            lg_all = sp.tile([128, 2 * n_tiles], F32, tag="lg_all")
            nc.vector.tensor_copy(
                lg_all[:].rearrange("p (b t) -> p b t", b=2),
                g3[:, :, 0:n_tiles])
            m_ge = sp.tile([128, 2 * n_tiles], I32, tag="m_ge")
            nc.vector.tensor_scalar(out=m_ge[:], in0=lg_all[:],
                                    scalar1=lo_f[:, 0:1], scalar2=None,
                                    op0=OP.is_ge)
            m_lt = sp.tile([128, 2 * n_tiles], I32, tag="m_lt")
            nc.vector.tensor_scalar(out=m_lt[:], in0=lg_all[:],
                                    scalar1=hi_f[:, 0:1], scalar2=None,
                                    op0=OP.is_lt)
            mb = sp.tile([128, 2 * n_tiles], I32, tag="mb")
            nc.vector.tensor_tensor(out=mb[:], in0=m_ge[:], in1=m_lt[:],
                                    op=OP.mult)
            nvv = max(8, 2 * n_tiles)
            vv = sp.tile([128, nvv], F32, tag="vv")
            nc.vector.memset(vv[:], -3.0e38)
            nc.vector.copy_predicated(out=vv[:, 0:2 * n_tiles], mask=mb[:],
                                      data=lg_all[:])
            cands_f = sp.tile([128, 8], F32, tag="cands_f")
            nc.vector.max(cands_f[:], vv[:])

            # count of logits >= hi (above bracket), replicated total
            chi = sp.tile([128, 2 * n_tiles], F32, tag="chi")
            nc.vector.tensor_scalar(out=chi[:], in0=lg_all[:],
                                    scalar1=hi_f[:, 0:1], scalar2=None,
                                    op0=OP.is_ge)
            chicol = sp.tile([128, 1], F32, tag="chicol")
            nc.vector.tensor_reduce(out=chicol[:], in_=chi[:],
                                    axis=mybir.AxisListType.X, op=OP.add)
            ps_chi = psp.tile([128, 1], F32, tag="ps_chi")
            nc.tensor.matmul(ps_chi[:], ones_sb[:], chicol[:], start=True,
                             stop=True)
            m_tgt = sp.tile([128, 1], F32, tag="m_tgt")
            nc.vector.tensor_scalar(out=m_tgt[:], in0=ps_chi[:], scalar1=-1.0,
                                    scalar2=float(k), op0=OP.mult, op1=OP.add)

            # --- 16-bit split monotone keys --------------------------------
            # DVE ALU is fp32 internally, so int add/mult above 2^24 are
            # lossy. Bitwise/shift ops are exact, and integers <= 2^16 are
            # exact in f32. Map f32 -> sortable uint32 with bitwise ops only
            # (ukey = bits ^ (bits<0 ? 0xFFFFFFFF : 0x80000000)), split into
            # hi/lo 16-bit halves stored as f32, then select the m-th
            # largest with two levels of 64-ary counting (fixed steps
            # 1024/16/1, all values f32-exact).
            def split_key(src_f32_ap, shp, tagp):
                bits = src_f32_ap.bitcast(I32)

                def cc(idx):
                    return intc[:, idx:idx + 1].broadcast_to(shp)

                s1 = sp.tile(shp, I32, tag=tagp + "_s1")
                nc.vector.tensor_tensor(out=s1[:], in0=bits, in1=cc(0),
                                        op=OP.arith_shift_right)  # 0 / -1
                nc.vector.tensor_tensor(out=s1[:], in0=s1[:], in1=cc(1),
                                        op=OP.bitwise_and)  # 0 / 0x7FFFFFFF
                nc.vector.tensor_tensor(out=s1[:], in0=s1[:], in1=cc(2),
                                        op=OP.bitwise_or)  # 0x8000.. / 0xFF..
                uk = sp.tile(shp, I32, tag=tagp + "_uk")
                nc.vector.tensor_tensor(out=uk[:], in0=bits, in1=s1[:],
                                        op=OP.bitwise_xor)
                hi_i = sp.tile(shp, I32, tag=tagp + "_hii")
                nc.vector.tensor_tensor(out=hi_i[:], in0=uk[:], in1=cc(3),
                                        op=OP.arith_shift_right)
                nc.vector.tensor_tensor(out=hi_i[:], in0=hi_i[:], in1=cc(4),
                                        op=OP.bitwise_and)
                lo_i = sp.tile(shp, I32, tag=tagp + "_loi")
                nc.vector.tensor_tensor(out=lo_i[:], in0=uk[:], in1=cc(4),
                                        op=OP.bitwise_and)
                hi_f = sp.tile(shp, F32, tag=tagp + "_hif")
                nc.vector.tensor_copy(hi_f[:], hi_i[:])
                lo16_f = sp.tile(shp, F32, tag=tagp + "_lof")
                nc.vector.tensor_copy(lo16_f[:], lo_i[:])
                return hi_f, lo16_f

            ch_f, cl_f = split_key(cands_f[:], [128, 8], "kc")

            # 64-ary counting selection: returns the m-th largest value of
            # vals (a [128, 8] f32 tile of integers in [-1, 65535]).
            def select16(vals, m_col, tagp):
                vals_b = vals[:].rearrange("p (e c) -> p e c",
                                           e=1).broadcast_to([128, 64, 8])
                lo = sp.tile([128, 1], F32, tag=tagp + "_lo")
                nc.vector.memset(lo[:], 0.0)
                for r, stp in enumerate((1024.0, 16.0, 1.0)):
                    edges = sp.tile([128, 64], F32, tag=tagp + "_ed")
                    nc.vector.tensor_scalar(
                        out=edges[:], in0=iota64f[:], scalar1=stp,
                        scalar2=lo[:, 0:1], op0=OP.mult, op1=OP.add)
                    cmp = sp.tile([128, 64 * 8], F32, tag=tagp + "_cmp")
                    c3v = cmp[:].rearrange("p (e c) -> p e c", e=64)
                    nc.vector.tensor_tensor(
                        out=c3v, in0=vals_b,
                        in1=edges[:].rearrange(
                            "p (e c) -> p e c", c=1).broadcast_to(
                                [128, 64, 8]),
                        op=OP.is_ge)
                    cntpp = sp.tile([128, 64], F32, tag=tagp + "_cnt")
                    nc.vector.tensor_reduce(out=cntpp[:], in_=c3v,
                                            axis=mybir.AxisListType.X,
                                            op=OP.add)
                    ps_c2 = psp.tile([128, 64], F32, tag="ps_c2")
                    nc.tensor.matmul(ps_c2[:], ones_sb[:], cntpp[:],
                                     start=True, stop=True)
                    ind2 = sp.tile([128, 64], F32, tag=tagp + "_ind")
                    nc.vector.tensor_scalar(out=ind2[:], in0=ps_c2[:],
                                            scalar1=m_col[:, 0:1],
                                            scalar2=None, op0=OP.is_ge)
                    sel2 = sp.tile([128, 64], F32, tag=tagp + "_sel")
                    nc.vector.tensor_tensor(out=sel2[:], in0=ind2[:],
                                            in1=iota64f[:], op=OP.mult)
                    jr = sp.tile([128, 1], F32, tag=tagp + "_jr")
                    nc.vector.tensor_reduce(out=jr[:], in_=sel2[:],
                                            axis=mybir.AxisListType.X,
                                            op=OP.max)
                    lo_new = sp.tile([128, 1], F32, tag=tagp + "_lo")
                    nc.vector.tensor_scalar(out=lo_new[:], in0=jr[:],
                                            scalar1=stp,
                                            scalar2=lo[:, 0:1],
                                            op0=OP.mult, op1=OP.add)
                    lo = lo_new
                return lo

            t_hi = select16(ch_f, m_tgt, "l1")

            # m2 = m_tgt - count(hi > t_hi)
            cgt = sp.tile([128, 8], F32, tag="cgt")
            nc.vector.tensor_scalar(out=cgt[:], in0=ch_f[:],
                                    scalar1=t_hi[:, 0:1], scalar2=None,
                                    op0=OP.is_gt)
            cgtc = sp.tile([128, 1], F32, tag="cgtc")
            nc.vector.tensor_reduce(out=cgtc[:], in_=cgt[:],
                                    axis=mybir.AxisListType.X, op=OP.add)
            ps_cgt = psp.tile([128, 1], F32, tag="ps_cgt")
            nc.tensor.matmul(ps_cgt[:], ones_sb[:], cgtc[:], start=True,
                             stop=True)
            m2 = sp.tile([128, 1], F32, tag="m2")
            nc.vector.tensor_scalar(out=m2[:], in0=ps_cgt[:], scalar1=-1.0,
                                    scalar2=m_tgt[:, 0:1], op0=OP.mult,
                                    op1=OP.add)

            # level 2: lo16 among hi == t_hi (others -> -1)
            eqh = sp.tile([128, 8], F32, tag="eqh")
            nc.vector.tensor_scalar(out=eqh[:], in0=ch_f[:],
                                    scalar1=t_hi[:, 0:1], scalar2=None,
                                    op0=OP.is_equal)
            lom = sp.tile([128, 8], F32, tag="lom")
            nc.vector.tensor_scalar(out=lom[:], in0=cl_f[:], scalar1=1.0,
                                    scalar2=None, op0=OP.add)
            nc.vector.tensor_tensor(out=lom[:], in0=lom[:], in1=eqh[:],
                                    op=OP.mult)
            nc.vector.tensor_scalar(out=lom[:], in0=lom[:], scalar1=-1.0,
                                    scalar2=None, op0=OP.add)
            t_lo = select16(lom, m2, "l2")

            # --- outputs ---------------------------------------------------
            sh_f, sl_f = split_key(logits_sb[:], [128, n_tiles], "ks")
            mgt_s = sp.tile([128, n_tiles], F32, tag="mgt_s")
            nc.vector.tensor_scalar(out=mgt_s[:], in0=sh_f[:],
                                    scalar1=t_hi[:, 0:1], scalar2=None,
                                    op0=OP.is_gt)
            meq_s = sp.tile([128, n_tiles], F32, tag="meq_s")
            nc.vector.tensor_scalar(out=meq_s[:], in0=sh_f[:],
                                    scalar1=t_hi[:, 0:1], scalar2=None,
                                    op0=OP.is_equal)
            mlo_s = sp.tile([128, n_tiles], F32, tag="mlo_s")
            nc.vector.tensor_scalar(out=mlo_s[:], in0=sl_f[:],
                                    scalar1=t_lo[:, 0:1], scalar2=None,
                                    op0=OP.is_ge)
            nc.vector.tensor_tensor(out=meq_s[:], in0=meq_s[:], in1=mlo_s[:],
                                    op=OP.mult)
            mask_sb = sp.tile([128, n_tiles], F32, tag="mask_sb")
            nc.vector.tensor_tensor(out=mask_sb[:], in0=mgt_s[:],
                                    in1=meq_s[:], op=OP.add)

            total = sp.tile([128, 1], F32, tag="total")
            nc.vector.tensor_tensor(out=total[:], in0=g3[:, 0, 32:33],
                                    in1=g3[:, 1, 32:33], op=OP.add)
            xdev = sp.tile([128, 1], F32, tag="xdev")
            nc.vector.tensor_scalar(out=xdev[:], in0=total[:],
                                    scalar1=1.0 / (2 * s_loc), scalar2=-0.5,
                                    op0=OP.mult, op1=OP.add)
            xsq = sp.tile([128, 1], F32, tag="xsq")
            nc.scalar.activation(xsq[:], xdev[:], ACT.Square)
            aux_sb = sp.tile([128, 1], F32, tag="aux_sb")
            nc.vector.tensor_scalar_mul(out=aux_sb[:], in0=xsq[:],
                                        scalar1=0.01)

            if debug_dump:
                dbg = nc.dram_tensor("dbg", [1, 16], F32,
                                     kind="ExternalOutput")
                dbg2 = nc.dram_tensor("dbg2", [1, N_HIST], F32,
                                      kind="ExternalOutput")
                dbg3 = nc.dram_tensor("dbg3", [1, 8], F32,
                                      kind="ExternalOutput")
                dbgrow = sp.tile([128, 16], F32, tag="dbgrow")
                nc.vector.memset(dbgrow[:], 0.0)
                nc.vector.tensor_copy(dbgrow[:, 0:1], lo_f[:])
                nc.vector.tensor_copy(dbgrow[:, 1:2], hi_f[:])
                nc.vector.tensor_copy(dbgrow[:, 2:3], ps_chi[:])
                nc.vector.tensor_copy(dbgrow[:, 3:4], m_tgt[:])
                nc.vector.tensor_copy(dbgrow[:, 4:5], jcol[:])
                nc.vector.tensor_copy(dbgrow[:, 5:6], t_hi[:])
                nc.vector.tensor_copy(dbgrow[:, 6:7], t_lo[:])
                nc.vector.tensor_copy(dbgrow[:, 7:8], m2[:])
                nc.sync.dma_start(out=dbg[:], in_=dbgrow[0:1, :])
                nc.sync.dma_start(out=dbg2[:], in_=hist_tot[0:1, :])
                nc.sync.dma_start(out=dbg3[:], in_=cands_f[0:1, :])
            nc.sync.dma_start(
                out=probs_out.ap().rearrange("(i p) -> p i", p=128),
                in_=probs_sb[:])
            nc.sync.dma_start(
                out=mask_out.ap().rearrange("(i p) -> p i", p=128),
                in_=mask_sb[:])
            nc.sync.dma_start(out=aux_out[:], in_=aux_sb[0:1, 0:1])

    nc.finalize()
    return nc


_NC_CACHE = {}


def _get_nc():
    if "nc" not in _NC_CACHE:
        _NC_CACHE["nc"] = build_kernel()
    return _NC_CACHE["nc"]


def make_in_maps(hidden, router_weight, router_bias):
    w_rep = np.ascontiguousarray(
        np.broadcast_to(np.asarray(router_weight, np.float32)[None, :],
                        (128, D)))
    b_rep = np.full((128, 1), float(router_bias), np.float32)
    in_maps = []
    for c in range(N_CORES):
        b, h = divmod(c, 2)
        shard = np.ascontiguousarray(
            np.asarray(hidden, np.float32)[b, h * S_LOC:(h + 1) * S_LOC, :])
        in_maps.append({"hid": shard, "w_rep": w_rep, "bias_rep": b_rep})
    return in_maps


def assemble(results):
    probs = np.zeros((B, S), np.float32)
    mask = np.zeros((B, S), np.float32)
    aux = np.zeros((B,), np.float32)
    for c in range(N_CORES):
        b, h = divmod(c, 2)
        probs[b, h * S_LOC:(h + 1) * S_LOC] = results[c]["probs"]
        mask[b, h * S_LOC:(h + 1) * S_LOC] = results[c]["mask"]
        if h == 0:
            aux[b] = results[c]["aux"][0, 0]
    return probs, mask, aux


def kernel(hidden, router_weight, router_bias):
    nc = _get_nc()
    in_maps = make_in_maps(hidden, router_weight, router_bias)
    res = run_bass_kernel_spmd(nc, in_maps, core_ids=list(range(N_CORES)))
    return assemble(res.results)


if __name__ == "__main__":
    nc = build_kernel()
    print("kernel built ok")
